# revision 11
# baseline (speedup 1.0000x reference)
"""Trainium2 Bass kernel for nn_CenterContrastiveLoss (fp8 screen version).

Problem: loss = label-smoothed CE over [pos, top-50 negs] of f @ centers.T
  f: [2048, 256] f32, centers: [65536, 256] f32, label: [2048] int.

Strategy (8 NeuronCores, tensor-parallel over C=65536):
  - Scores are computed in fp8-e4m3 DoubleRow matmuls: K=256 packed as
    2x128 (d-halves), one MM per 512-column chunk, 16 MMs per row-tile
    sharing one LDWEIGHTS (rt-outer loop).
  - PSUM tiles are [128 x 1024] (2 banks) x 4 buffers, so bank turnover
    stays off the critical path.
  - Eviction is split by subtile between the two PSUM-capable engines:
    ScalarE tiles: one Exp activation PSUM->bf16 SBUF, then DMA the exp
      values to HBM (host reduces them exactly - no on-device fold).
    VectorE tiles: one grouped 16:1 max-reduce PSUM->f16 (bucket maxima).
    Split 72/56 tiles per core to balance engine time (~70us each).
  - Host merges: se_negs from exact exp sums (scalar share) + exp of
    fine maxima (vector share); top-50 values from 16-wide bucket maxima
    of both shares; positive removed analytically (its tile/bucket is
    known from label); loss = mean(0.9102*lse - 0.9002*pos - 0.0002*S1).
    fp8 score noise (sigma ~0.6) keeps final rel err ~7e-4.
"""

import numpy as np
import ml_dtypes

B, C, D = 2048, 65536, 256
NCORES = 8
CSH = C // NCORES          # 8192
RT = B // 128              # 16
NST = 8                    # 1024-wide subtiles per row-tile per core
STW = 1024
NCH = CSH // 512           # 16 512-col matmul chunks per core
SHIFT = 60.0
FP8 = ml_dtypes.float8_e4m3

_prog = None


def _is_scalar(rt, st):
    return (st % 2 == 0) or (st == 7 and rt in (1, 5, 9))


SCALAR_TILES = [(rt, st) for rt in range(RT) for st in range(NST)
                if _is_scalar(rt, st)]
NSC = len(SCALAR_TILES)    # 72
SC_IDX = {t: i for i, t in enumerate(SCALAR_TILES)}


def _vector_sts(rt):
    return [st for st in range(NST) if not _is_scalar(rt, st)]


def _build_program():
    import concourse.mybir as mybir
    from concourse import bacc
    from concourse.tile import TileContext
    from contextlib import ExitStack

    fp8 = mybir.dt.float8e4
    bf16 = mybir.dt.bfloat16
    f16 = mybir.dt.float16
    f32 = mybir.dt.float32
    DR = mybir.MatmulPerfMode.DoubleRow

    nc = bacc.Bacc("TRN2")
    # fT free layout: rt*256 + h*128 + r   (h = d-half, r = row-in-tile)
    fT_d = nc.declare_dram_parameter("fT", [128, RT * 256], fp8, isOutput=False)
    # cT free layout: chunk*1024 + h*512 + c
    cT_d = nc.declare_dram_parameter("cT", [128, CSH * 2], fp8, isOutput=False)
    exp_d = nc.declare_dram_parameter("out_exp", [NSC, 128, STW], bf16,
                                      isOutput=True)
    fine_d = nc.declare_dram_parameter("out_fine", [RT, 128, 256], f16,
                                       isOutput=True)

    with TileContext(nc) as tc, ExitStack() as ctx:
        const = ctx.enter_context(tc.tile_pool(name="const", bufs=1))
        psum_s = ctx.enter_context(tc.tile_pool(name="psum_s", bufs=2,
                                                space="PSUM"))
        psum_v = ctx.enter_context(tc.tile_pool(name="psum_v", bufs=2,
                                                space="PSUM"))
        scr = ctx.enter_context(tc.tile_pool(name="scr", bufs=4))
        finep = ctx.enter_context(tc.tile_pool(name="finep", bufs=3))

        fT_t = const.tile([128, RT * 256], fp8, tag="fT", name="fT")
        cT_t = const.tile([128, CSH * 2], fp8, tag="cT", name="cT")
        bias_t = const.tile([128, 1], f32, tag="bias", name="bias")
        nc.vector.memset(bias_t[:], -SHIFT)

        # input DMAs in consumption order, split across two queues
        nc.sync.dma_start(out=fT_t[:, 0:512], in_=fT_d[:, 0:512])
        for ch in range(NCH):
            eng = nc.sync if ch % 2 == 0 else nc.scalar
            eng.dma_start(out=cT_t[:, ch * 1024:(ch + 1) * 1024],
                          in_=cT_d[:, ch * 1024:(ch + 1) * 1024])
            if ch == 3:
                nc.scalar.dma_start(out=fT_t[:, 512:RT * 256],
                                    in_=fT_d[:, 512:RT * 256])

        for rt in range(RT):
            lhsT = fT_t[:, rt * 256:(rt + 1) * 256].rearrange(
                "p (h r) -> p h r", h=2)
            vst = _vector_sts(rt)
            vw = len(vst) * 64
            fine_sb = finep.tile([128, 256], f16, tag="fine", name="fine_sb")
            for st in range(NST):
                is_sc = _is_scalar(rt, st)
                pool = psum_s if is_sc else psum_v
                tag = "pts" if is_sc else "ptv"
                pt = pool.tile([128, STW], f32, tag=tag, name=tag)
                for n in range(2):
                    ch = st * 2 + n
                    rhs = cT_t[:, ch * 1024:(ch + 1) * 1024].rearrange(
                        "p (h c) -> p h c", h=2)
                    nc.tensor.matmul(pt[:, n * 512:(n + 1) * 512], lhsT, rhs,
                                     start=True, stop=True, perf_mode=DR)
                if is_sc:
                    et = scr.tile([128, STW], bf16, tag="et", name="et")
                    nc.scalar.activation(
                        out=et[:],
                        in_=pt[:],
                        func=mybir.ActivationFunctionType.Exp,
                        bias=bias_t[:],
                        scale=1.0,
                    )
                    k = SC_IDX[(rt, st)]
                    eng = nc.scalar if k % 2 == 0 else nc.sync
                    eng.dma_start(out=exp_d[k], in_=et[:])
                else:
                    j = vst.index(st)
                    nc.vector.tensor_reduce(
                        out=fine_sb[:, j * 64:(j + 1) * 64],
                        in_=pt[:].rearrange("p (g e) -> p g e", e=16),
                        axis=mybir.AxisListType.X,
                        op=mybir.AluOpType.max,
                    )
            nc.sync.dma_start(out=fine_d[rt, :, 0:vw], in_=fine_sb[:, 0:vw])

    nc.finalize()
    return nc


def _get_program():
    global _prog
    if _prog is None:
        _prog = _build_program()
    return _prog


def run_device(in_maps, trace=False, **kw):
    from concourse.bass_utils import run_bass_kernel_spmd

    nc = _get_program()
    return run_bass_kernel_spmd(nc, in_maps, core_ids=list(range(NCORES)),
                                trace=trace, **kw)


def make_in_maps(f, centers, label):
    fq = np.asarray(f, dtype=np.float32).astype(FP8)
    fT = np.ascontiguousarray(
        fq.reshape(RT, 128, 2, 128).transpose(3, 0, 2, 1)).reshape(128, RT * 256)
    cq = np.asarray(centers, dtype=np.float32).astype(FP8)
    in_maps = []
    for core in range(NCORES):
        cs = cq[core * CSH:(core + 1) * CSH]
        cT = np.ascontiguousarray(
            cs.reshape(NCH, 512, 2, 128).transpose(3, 0, 2, 1)).reshape(
                128, CSH * 2)
        in_maps.append({"fT": fT, "cT": cT})
    return in_maps


def postprocess(results, f, centers, label):
    rows = np.arange(B)

    # positive score as the device computed it (fp8 inputs, f32 accumulate
    # per d-half), and exactly (f64) for the loss formula
    fq = np.asarray(f, dtype=np.float32).astype(FP8).astype(np.float32)
    cq = np.asarray(centers, dtype=np.float32).astype(FP8).astype(np.float32)
    pc = cq[label]
    pos_sim = (np.sum(fq[:, :128] * pc[:, :128], axis=1, dtype=np.float32)
               + np.sum(fq[:, 128:] * pc[:, 128:], axis=1,
                        dtype=np.float32)).astype(np.float64)
    pos_exact = np.einsum("ij,ij->i", np.asarray(f, dtype=np.float64),
                          np.asarray(centers, dtype=np.float64)[label])

    lab = np.asarray(label)
    core_p = lab // CSH
    c_in = lab % CSH
    st_p = c_in // STW
    rt_p = rows // 128
    in_scalar = np.array([_is_scalar(int(rt), int(st))
                          for rt, st in zip(rt_p, st_p)])
    pe_ = np.exp(pos_sim - SHIFT)

    # map (rt, st) -> scalar tile index / vector j
    sc_idx_arr = -np.ones((RT, NST), dtype=np.int64)
    vj_arr = -np.ones((RT, NST), dtype=np.int64)
    for rt in range(RT):
        for st in range(NST):
            if _is_scalar(rt, st):
                sc_idx_arr[rt, st] = SC_IDX[(rt, st)]
            else:
                vj_arr[rt, st] = _vector_sts(rt).index(st)

    se = np.zeros(B)
    cand_parts = []
    for core, r in enumerate(results):
        ev = np.asarray(r["out_exp"], dtype=ml_dtypes.bfloat16).astype(
            np.float32)                       # [NSC, 128, 1024]
        # exact per-tile sums and 16-wide bucket maxima
        tile_sum = ev.sum(axis=2, dtype=np.float64)        # [NSC, 128]
        bmax = ev.reshape(NSC, 128, 64, 16).max(axis=3)    # [NSC, 128, 64]

        # positive removal for rows whose positive sits in this core's
        # scalar share
        m = in_scalar & (core_p == core)
        if m.any():
            ridx = rows[m]
            k = sc_idx_arr[rt_p[m], st_p[m]]
            rr = ridx % 128
            tile_sum[k, rr] = np.maximum(tile_sum[k, rr] - pe_[m], 1e-30)
            bidx = (c_in[m] % STW) // 16
            bv = bmax[k, rr, bidx]
            hit = np.abs(bv - pe_[m]) <= np.maximum(0.01 * pe_[m], 1e-12)
            bmax[k[hit], rr[hit], bidx[hit]] = 0.0

        # scatter per-tile results back to rows
        sums_rows = np.zeros((B, NSC // RT + 1))
        cand_sc = np.full((B, (NSC // RT + 1) * 64), 1e-300)
        slot = np.zeros(RT, dtype=np.int64)
        for k, (rt, st) in enumerate(SCALAR_TILES):
            sl = slot[rt]; slot[rt] += 1
            rsl = slice(rt * 128, (rt + 1) * 128)
            sums_rows[rsl, sl] = tile_sum[k]
            cand_sc[rsl, sl * 64:(sl + 1) * 64] = bmax[k]
        se += sums_rows.sum(axis=1)
        cand_parts.append(SHIFT + np.log(np.maximum(cand_sc, 1e-300)))

        fine = np.asarray(r["out_fine"], dtype=np.float16).astype(
            np.float64)                        # [RT, 128, 256]
        fine_rows = np.full((B, 256), -np.inf)
        for rt in range(RT):
            vw = len(_vector_sts(rt)) * 64
            fine_rows[rt * 128:(rt + 1) * 128, :vw] = fine[rt, :, :vw]
        # positive removal in the vector share
        m = (~in_scalar) & (core_p == core)
        if m.any():
            ridx = rows[m]
            j = vj_arr[rt_p[m], st_p[m]]
            fidx = j * 64 + (c_in[m] % STW) // 16
            bv = fine_rows[ridx, fidx]
            hit = np.abs(bv - pos_sim[m]) < 0.15
            fine_rows[ridx[hit], fidx[hit]] = -np.inf
        se += np.exp(fine_rows - SHIFT, where=np.isfinite(fine_rows),
                     out=np.zeros_like(fine_rows)).sum(axis=1)
        cand_parts.append(fine_rows)

    cand = np.concatenate(cand_parts, axis=1)
    top50 = -np.partition(-cand, 49, axis=1)[:, :50]
    S1 = top50.sum(axis=1)
    lse = SHIFT + np.log(se + np.exp(pos_exact - SHIFT))
    loss = (0.9102 * lse - 0.9002 * pos_exact - 0.0002 * S1).mean()
    return np.array(loss, dtype=np.float32)


def kernel(f, centers, label):
    f = np.asarray(f, dtype=np.float32)
    centers = np.asarray(centers, dtype=np.float32)
    label = np.asarray(label).astype(np.int64)
    in_maps = make_in_maps(f, centers, label)
    try:
        res = run_device(in_maps)
    except Exception:
        # transient runtime flakes (e.g. NRT_EXEC_UNIT_UNRECOVERABLE) have
        # been observed to succeed on immediate retry
        res = run_device(in_maps)
    return postprocess(res.results, f, centers, label)


# revision 13
# speedup vs baseline: 1.2619x; 1.2619x over previous
"""Trainium2 Bass kernel for nn_CenterContrastiveLoss (fp8 screen version).

Problem: loss = label-smoothed CE over [pos, top-50 negs] of f @ centers.T
  f: [2048, 256] f32, centers: [65536, 256] f32, label: [2048] int.

Strategy (8 NeuronCores, tensor-parallel over C=65536):
  - Scores are computed in fp8-e4m3 DoubleRow matmuls: K=256 packed as
    2x128 (d-halves), one MM per 512-column chunk, 16 MMs per row-tile
    sharing one LDWEIGHTS (rt-outer loop).
  - PSUM tiles are [128 x 1024] (2 banks) x 4 buffers, so bank turnover
    stays off the critical path.
  - Eviction is split by subtile between the two PSUM-capable engines:
    ScalarE tiles: one Exp activation PSUM->bf16 SBUF, then DMA the exp
      values to HBM (host reduces them exactly - no on-device fold).
    VectorE tiles: one grouped 16:1 max-reduce PSUM->f16 (bucket maxima).
    Split 72/56 tiles per core to balance engine time (~70us each).
  - Host merges: se_negs from exact exp sums (scalar share) + exp of
    fine maxima (vector share); top-50 values from 16-wide bucket maxima
    of both shares; positive removed analytically (its tile/bucket is
    known from label); loss = mean(0.9102*lse - 0.9002*pos - 0.0002*S1).
    fp8 score noise (sigma ~0.6) keeps final rel err ~7e-4.
"""

import numpy as np
import ml_dtypes

B, C, D = 2048, 65536, 256
NCORES = 8
CSH = C // NCORES          # 8192
RT = B // 128              # 16
NST = 8                    # 1024-wide subtiles per row-tile per core
STW = 1024
NCH = CSH // 512           # 16 512-col matmul chunks per core
SHIFT = 60.0
FP8 = ml_dtypes.float8_e4m3

_prog = None


def _is_scalar(rt, st):
    return (st % 2 == 0) or (st == 7 and rt in (1, 5, 9))


SCALAR_TILES = [(rt, st) for rt in range(RT) for st in range(NST)
                if _is_scalar(rt, st)]
NSC = len(SCALAR_TILES)    # 72
SC_IDX = {t: i for i, t in enumerate(SCALAR_TILES)}


def _vector_sts(rt):
    return [st for st in range(NST) if not _is_scalar(rt, st)]


def _build_program():
    import concourse.mybir as mybir
    from concourse import bacc
    from concourse.tile import TileContext
    from contextlib import ExitStack

    fp8 = mybir.dt.float8e4
    bf16 = mybir.dt.bfloat16
    f16 = mybir.dt.float16
    f32 = mybir.dt.float32
    DR = mybir.MatmulPerfMode.DoubleRow

    nc = bacc.Bacc("TRN2")
    # fT free layout: rt*256 + h*128 + r   (h = d-half, r = row-in-tile)
    fT_d = nc.declare_dram_parameter("fT", [128, RT * 256], fp8, isOutput=False)
    # cT free layout: chunk*1024 + h*512 + c
    cT_d = nc.declare_dram_parameter("cT", [128, CSH * 2], fp8, isOutput=False)
    exp_d = nc.declare_dram_parameter("out_exp", [NSC, 128, STW], bf16,
                                      isOutput=True)
    fine_d = nc.declare_dram_parameter("out_fine", [RT, 128, 256], f16,
                                       isOutput=True)

    with TileContext(nc) as tc, ExitStack() as ctx:
        const = ctx.enter_context(tc.tile_pool(name="const", bufs=1))
        psum_s = ctx.enter_context(tc.tile_pool(name="psum_s", bufs=2,
                                                space="PSUM"))
        psum_v = ctx.enter_context(tc.tile_pool(name="psum_v", bufs=2,
                                                space="PSUM"))
        scr = ctx.enter_context(tc.tile_pool(name="scr", bufs=4))
        finep = ctx.enter_context(tc.tile_pool(name="finep", bufs=3))

        fT_t = const.tile([128, RT * 256], fp8, tag="fT", name="fT")
        cT_t = const.tile([128, CSH * 2], fp8, tag="cT", name="cT")
        bias_t = const.tile([128, 1], f32, tag="bias", name="bias")
        nc.vector.memset(bias_t[:], -SHIFT)

        # input DMAs in consumption order, split across two queues; the
        # first chunks are scheduled "earliest" so the first MMs only wait
        # on them, later chunks get a staged logical timestamp
        nc.sync.dma_start(out=fT_t[:, 0:512], in_=fT_d[:, 0:512])
        for ch in range(4):
            eng = nc.sync if ch % 2 == 0 else nc.gpsimd
            eng.dma_start(out=cT_t[:, ch * 1024:(ch + 1) * 1024],
                          in_=cT_d[:, ch * 1024:(ch + 1) * 1024])
        with tc.tile_wait_until(0.002):
            for ch in range(4, NCH):
                eng = nc.sync if ch % 2 == 0 else nc.gpsimd
                eng.dma_start(out=cT_t[:, ch * 1024:(ch + 1) * 1024],
                              in_=cT_d[:, ch * 1024:(ch + 1) * 1024])
                if ch == 5:
                    nc.gpsimd.dma_start(out=fT_t[:, 512:RT * 256],
                                        in_=fT_d[:, 512:RT * 256])

        for rt in range(RT):
            lhsT = fT_t[:, rt * 256:(rt + 1) * 256].rearrange(
                "p (h r) -> p h r", h=2)
            vst = _vector_sts(rt)
            vw = len(vst) * 64
            fine_sb = finep.tile([128, 256], f16, tag="fine", name="fine_sb")
            for st in range(NST):
                is_sc = _is_scalar(rt, st)
                pool = psum_s if is_sc else psum_v
                tag = "pts" if is_sc else "ptv"
                pt = pool.tile([128, STW], f32, tag=tag, name=tag)
                for n in range(2):
                    ch = st * 2 + n
                    rhs = cT_t[:, ch * 1024:(ch + 1) * 1024].rearrange(
                        "p (h c) -> p h c", h=2)
                    nc.tensor.matmul(pt[:, n * 512:(n + 1) * 512], lhsT, rhs,
                                     start=True, stop=True, perf_mode=DR)
                if is_sc:
                    et = scr.tile([128, STW], bf16, tag="et", name="et")
                    nc.scalar.activation(
                        out=et[:],
                        in_=pt[:],
                        func=mybir.ActivationFunctionType.Exp,
                        bias=bias_t[:],
                        scale=1.0,
                    )
                    k = SC_IDX[(rt, st)]
                    nc.sync.dma_start(out=exp_d[k], in_=et[:])
                else:
                    j = vst.index(st)
                    nc.vector.tensor_reduce(
                        out=fine_sb[:, j * 64:(j + 1) * 64],
                        in_=pt[:].rearrange("p (g e) -> p g e", e=16),
                        axis=mybir.AxisListType.X,
                        op=mybir.AluOpType.max,
                    )
            nc.sync.dma_start(out=fine_d[rt, :, 0:vw], in_=fine_sb[:, 0:vw])

    nc.finalize()
    return nc


def _get_program():
    global _prog
    if _prog is None:
        _prog = _build_program()
    return _prog


def run_device(in_maps, trace=False, **kw):
    from concourse.bass_utils import run_bass_kernel_spmd

    nc = _get_program()
    return run_bass_kernel_spmd(nc, in_maps, core_ids=list(range(NCORES)),
                                trace=trace, **kw)


def make_in_maps(f, centers, label):
    fq = np.asarray(f, dtype=np.float32).astype(FP8)
    fT = np.ascontiguousarray(
        fq.reshape(RT, 128, 2, 128).transpose(3, 0, 2, 1)).reshape(128, RT * 256)
    cq = np.asarray(centers, dtype=np.float32).astype(FP8)
    in_maps = []
    for core in range(NCORES):
        cs = cq[core * CSH:(core + 1) * CSH]
        cT = np.ascontiguousarray(
            cs.reshape(NCH, 512, 2, 128).transpose(3, 0, 2, 1)).reshape(
                128, CSH * 2)
        in_maps.append({"fT": fT, "cT": cT})
    return in_maps


def postprocess(results, f, centers, label):
    rows = np.arange(B)

    # positive score as the device computed it (fp8 inputs, f32 accumulate
    # per d-half), and exactly (f64) for the loss formula
    fq = np.asarray(f, dtype=np.float32).astype(FP8).astype(np.float32)
    cq = np.asarray(centers, dtype=np.float32).astype(FP8).astype(np.float32)
    pc = cq[label]
    pos_sim = (np.sum(fq[:, :128] * pc[:, :128], axis=1, dtype=np.float32)
               + np.sum(fq[:, 128:] * pc[:, 128:], axis=1,
                        dtype=np.float32)).astype(np.float64)
    pos_exact = np.einsum("ij,ij->i", np.asarray(f, dtype=np.float64),
                          np.asarray(centers, dtype=np.float64)[label])

    lab = np.asarray(label)
    core_p = lab // CSH
    c_in = lab % CSH
    st_p = c_in // STW
    rt_p = rows // 128
    in_scalar = np.array([_is_scalar(int(rt), int(st))
                          for rt, st in zip(rt_p, st_p)])
    pe_ = np.exp(pos_sim - SHIFT)

    # map (rt, st) -> scalar tile index / vector j
    sc_idx_arr = -np.ones((RT, NST), dtype=np.int64)
    vj_arr = -np.ones((RT, NST), dtype=np.int64)
    for rt in range(RT):
        for st in range(NST):
            if _is_scalar(rt, st):
                sc_idx_arr[rt, st] = SC_IDX[(rt, st)]
            else:
                vj_arr[rt, st] = _vector_sts(rt).index(st)

    se = np.zeros(B)
    cand_parts = []
    for core, r in enumerate(results):
        ev = np.asarray(r["out_exp"], dtype=ml_dtypes.bfloat16).astype(
            np.float32)                       # [NSC, 128, 1024]
        # exact per-tile sums and 16-wide bucket maxima
        tile_sum = ev.sum(axis=2, dtype=np.float64)        # [NSC, 128]
        bmax = ev.reshape(NSC, 128, 64, 16).max(axis=3)    # [NSC, 128, 64]

        # positive removal for rows whose positive sits in this core's
        # scalar share
        m = in_scalar & (core_p == core)
        if m.any():
            ridx = rows[m]
            k = sc_idx_arr[rt_p[m], st_p[m]]
            rr = ridx % 128
            tile_sum[k, rr] = np.maximum(tile_sum[k, rr] - pe_[m], 1e-30)
            bidx = (c_in[m] % STW) // 16
            bv = bmax[k, rr, bidx]
            hit = np.abs(bv - pe_[m]) <= np.maximum(0.01 * pe_[m], 1e-12)
            bmax[k[hit], rr[hit], bidx[hit]] = 0.0

        # scatter per-tile results back to rows
        sums_rows = np.zeros((B, NSC // RT + 1))
        cand_sc = np.full((B, (NSC // RT + 1) * 64), 1e-300)
        slot = np.zeros(RT, dtype=np.int64)
        for k, (rt, st) in enumerate(SCALAR_TILES):
            sl = slot[rt]; slot[rt] += 1
            rsl = slice(rt * 128, (rt + 1) * 128)
            sums_rows[rsl, sl] = tile_sum[k]
            cand_sc[rsl, sl * 64:(sl + 1) * 64] = bmax[k]
        se += sums_rows.sum(axis=1)
        cand_parts.append(SHIFT + np.log(np.maximum(cand_sc, 1e-300)))

        fine = np.asarray(r["out_fine"], dtype=np.float16).astype(
            np.float64)                        # [RT, 128, 256]
        fine_rows = np.full((B, 256), -np.inf)
        for rt in range(RT):
            vw = len(_vector_sts(rt)) * 64
            fine_rows[rt * 128:(rt + 1) * 128, :vw] = fine[rt, :, :vw]
        # positive removal in the vector share
        m = (~in_scalar) & (core_p == core)
        if m.any():
            ridx = rows[m]
            j = vj_arr[rt_p[m], st_p[m]]
            fidx = j * 64 + (c_in[m] % STW) // 16
            bv = fine_rows[ridx, fidx]
            hit = np.abs(bv - pos_sim[m]) < 0.15
            fine_rows[ridx[hit], fidx[hit]] = -np.inf
        se += np.exp(fine_rows - SHIFT, where=np.isfinite(fine_rows),
                     out=np.zeros_like(fine_rows)).sum(axis=1)
        cand_parts.append(fine_rows)

    cand = np.concatenate(cand_parts, axis=1)
    top50 = -np.partition(-cand, 49, axis=1)[:, :50]
    S1 = top50.sum(axis=1)
    lse = SHIFT + np.log(se + np.exp(pos_exact - SHIFT))
    loss = (0.9102 * lse - 0.9002 * pos_exact - 0.0002 * S1).mean()
    return np.array(loss, dtype=np.float32)


def kernel(f, centers, label):
    f = np.asarray(f, dtype=np.float32)
    centers = np.asarray(centers, dtype=np.float32)
    label = np.asarray(label).astype(np.int64)
    in_maps = make_in_maps(f, centers, label)
    try:
        res = run_device(in_maps)
    except Exception:
        # transient runtime flakes (e.g. NRT_EXEC_UNIT_UNRECOVERABLE) have
        # been observed to succeed on immediate retry
        res = run_device(in_maps)
    return postprocess(res.results, f, centers, label)
